# revision 44
# baseline (speedup 1.0000x reference)
"""Differentiable rasterization kernel for Trainium2 (8 NeuronCores, SPMD).

Problem: polygons (4,16,8,2) f32 in [0,1) -> out (4,16,256,256) f32
  out = sigmoid(C * D / (TAU*SIZE)) where
    D(i,j)  = sum_v ||(i,j) - vertex_v||           (sum of vertex distances)
    C(i,j)  = +1 inside polygon else -1            (even-odd rule)

Sharding: data-parallel over the 64 fused polygons, 8 per core.

Device-side strategy (per polygon image, [128 part = row%128, 512 free = (half,col)]):
  - PE (fp32r):  r2_v(i,j) = (i-vr)^2 + (j-vc)^2 as a K=3 matmul -> PSUM
  - ACT:         d_v = sqrt(r2_v)  PSUM->SBUF
  - PE (fp32r):  D += d_v  via identity-matmul accumulation into PSUM
  - PE (bf16):   term_e(i,j) = (n_e(j)+1) - i  (exact integers) -> PSUM
  - DVE:         parity ^= (term_e > 0)  fused scalar_tensor_tensor
  - GPSIMD:      arg = (2*parity-1) * D
  - ACT:         out = sigmoid(arg/512)   (single Sqrt->Sigmoid table switch)

Host-side prep is O(polygon-count): scanline crossing rows m_e(j) per column
(mirrors the reference f32 arithmetic), n = ceil(m)-1 so that the integer
comparison n+1 > i is exactly [m > i].
"""

import numpy as np
import ml_dtypes

import concourse.bass as bass
import concourse.bacc as bacc
import concourse.tile as tile
from concourse import mybir
from concourse.bass_utils import run_bass_kernel_spmd
from concourse._compat import get_trn_type

SIZE = 256
TAU = 2.0
NCORES = 8
IMGS = 8          # polygons per core
KV = 8            # vertices (= edges) per polygon
FD = 512          # free dim per image tile: (half h, col j)
ALU = mybir.AluOpType
AFT = mybir.ActivationFunctionType

_NC_CACHE = None


def _host_tables(polygons: np.ndarray):
    """Build per-core input tables. polygons: (4,16,8,2) f32."""
    b, n, k, _ = polygons.shape
    bn = b * n
    assert bn == NCORES * IMGS and k == KV
    # mirror reference: roll swaps (x,y)->(y,x), scale to pixels, f32 ops
    polys = (np.roll(polygons.astype(np.float32), 1, axis=-1)
             * np.float32(SIZE)).reshape(bn, k, 2)
    vr = polys[:, :, 0]            # row coords  (bn, k)
    vc = polys[:, :, 1]            # col coords  (bn, k)

    p = np.arange(128, dtype=np.float32)            # partition rows
    j = np.arange(SIZE, dtype=np.float32)           # columns
    h = np.array([0.0, 1.0], dtype=np.float32)      # row-half

    # ---- scanline crossings m_e(j): mirror reference _inside arithmetic ----
    x = vr                                           # (bn,k) edge start row
    y = vc                                           # (bn,k) edge start col
    xn = np.roll(vr, -1, axis=1)
    yn = np.roll(vc, -1, axis=1)
    pyj = j[None, None, :]                           # (1,1,256)
    cond = (y[:, :, None] > pyj) != (yn[:, :, None] > pyj)
    denom = np.where(cond, (yn - y)[:, :, None], np.float32(1.0)).astype(np.float32)
    # reference: xint = (xn - x) * (py - y) / denom + x  (f32 elementwise)
    num = ((xn - x)[:, :, None] * (pyj - y[:, :, None])).astype(np.float32)
    xint = (num / denom + x[:, :, None]).astype(np.float32)
    m = np.where(cond, xint, np.float32(-1e4))       # (bn, k, 256)
    # integer crossing row: [m > i]  <=>  [ceil(m)-1 >= i]  (exact, all m)
    nrow = np.clip(np.ceil(m) - 1.0, -1.0, 255.0).astype(np.float32)  # (bn,k,256)

    # constant rhs for the r2 matmul, centered to shrink fp32r rounding:
    # free layout f=(h,j), rows [(j-128)^2; (j-128); 1; w=128h-64]
    rhs_c = np.zeros((4, FD), dtype=np.float32)
    jc = j - np.float32(128.0)
    for hh in range(2):
        sl = slice(256 * hh, 256 * (hh + 1))
        rhs_c[0, sl] = jc * jc
        rhs_c[1, sl] = jc
        rhs_c[2, sl] = 1.0
        rhs_c[3, sl] = 128.0 * hh - 64.0
    icol = p.reshape(128, 1).copy()                  # per-partition row index

    in_maps = []
    for c in range(NCORES):
        g0 = c * IMGS
        # lhsT_r2[mi]: K=4 rows per vertex v, centered decomposition:
        #   r2 = (j-128)^2 - 2(vc-128)(j-128)
        #      + [(vc-128)^2 + (p+64-vr)^2 + 4096] + 2(p+64-vr)*(128h-64)
        #      = (j-vc)^2 + (p+128h-vr)^2
        lhsT_r2 = np.zeros((IMGS, 4, KV * 128), dtype=np.float32)
        # rhs_t[mi]: one q-row per edge: q(h,j) = n+1-128h (exact ints in bf16)
        rhs_t = np.zeros((IMGS, KV, FD), dtype=np.float32)
        for mi in range(IMGS):
            g = g0 + mi
            for v in range(KV):
                a = vc[g, v] - np.float32(128.0)
                u = (p + np.float32(64.0) - vr[g, v]).astype(np.float32)
                sl = slice(128 * v, 128 * (v + 1))
                lhsT_r2[mi, 0, sl] = 1.0
                lhsT_r2[mi, 1, sl] = np.float32(-2.0) * a
                lhsT_r2[mi, 2, sl] = a * a + u * u + np.float32(4096.0)
                lhsT_r2[mi, 3, sl] = np.float32(2.0) * u
                q = nrow[g, v] + 1.0
                for hh in range(2):
                    rhs_t[mi, v, 256 * hh:256 * (hh + 1)] = q - 128.0 * hh
        in_maps.append({
            "lhsT_r2": lhsT_r2,
            "rhs_t": rhs_t.astype(ml_dtypes.bfloat16),
            "rhs_c": rhs_c,
            "icol": icol,
            "ident": np.eye(128, dtype=np.float32).astype(ml_dtypes.bfloat16),
        })
    return in_maps


def _build_nc():
    f32 = mybir.dt.float32
    f32r = mybir.dt.float32r
    bf16 = mybir.dt.bfloat16

    nc = bacc.Bacc(
        get_trn_type(),
        target_bir_lowering=False,
        debug=False,
        num_devices=NCORES,
    )
    lhsT_r2_d = nc.declare_dram_parameter("lhsT_r2", [IMGS, 4, KV * 128], f32r, isOutput=False)
    rhs_t_d = nc.declare_dram_parameter("rhs_t", [IMGS, KV, FD], bf16, isOutput=False)
    rhs_c_d = nc.declare_dram_parameter("rhs_c", [4, FD], f32r, isOutput=False)
    icol_d = nc.declare_dram_parameter("icol", [128, 1], f32, isOutput=False)
    ident_d = nc.declare_dram_parameter("ident", [128, 128], bf16, isOutput=False)
    out_d = nc.declare_dram_parameter("out", [IMGS, 128, FD], f32, isOutput=True)

    with tile.TileContext(nc) as tc:
        with (
            tc.tile_pool(name="const", bufs=1) as cpool,
            tc.tile_pool(name="tabs", bufs=2) as tabpool,
            tc.tile_pool(name="r2", bufs=3, space="PSUM") as r2pool,      # 3x2 banks (pairs)
            tc.tile_pool(name="Dacc", bufs=2, space="PSUM") as dpool,     # 2 banks
            tc.tile_pool(name="qb", bufs=4) as qbpool,                    # SBUF bf16
            tc.tile_pool(name="dsb", bufs=3) as dsbpool,
            tc.tile_pool(name="cnt", bufs=2) as cntpool,
            tc.tile_pool(name="Dsb", bufs=2) as Dsbpool,
            tc.tile_pool(name="u", bufs=2) as upool,
            tc.tile_pool(name="arg", bufs=IMGS) as argpool,
            tc.tile_pool(name="osb", bufs=3) as opool,
        ):
            rhs_c = cpool.tile([4, FD], f32r)
            icol = cpool.tile([128, 1], f32)
            ident = cpool.tile([128, 128], bf16)
            sqbias = cpool.tile([128, 1], f32)
            nc.sync.dma_start(rhs_c[:], rhs_c_d[:])
            nc.sync.dma_start(icol[:], icol_d[:])
            nc.sync.dma_start(ident[:], ident_d[:])
            nc.gpsimd.memset(sqbias[:], 8.0)

            args = []
            sqrt_insts = []
            for mi in range(IMGS):
                lhsT_r2 = tabpool.tile([4, KV * 128], f32r, tag="lhsT_r2")
                nc.sync.dma_start(lhsT_r2[:], lhsT_r2_d[mi])
                D = dpool.tile([128, FD], f32)
                cnt = cntpool.tile([128, FD], f32, tag="cntA")
                # pair vertices: two r2 matmuls fill one [128,1024] PSUM tile,
                # one wide sqrt per pair; D-accumulation matmuls are emitted
                # one pair late so PE never head-of-line blocks on ACT.
                dsb_prev = None
                for ep in range(KV // 2):
                    r2 = r2pool.tile([128, 2 * FD], f32)
                    dsb = dsbpool.tile([128, 2 * FD], bf16)
                    for hi in range(2):
                        e = 2 * ep + hi
                        nc.tensor.matmul(
                            r2[:, FD * hi:FD * (hi + 1)],
                            lhsT_r2[:, 128 * e:128 * (e + 1)],
                            rhs_c[:],
                            start=True, stop=True,
                        )
                    # bias guards fp32r matmul rounding driving r2 slightly
                    # negative near a vertex (sqrt(neg)=NaN)
                    si = nc.scalar.activation(dsb[:], r2[:], AFT.Sqrt, bias=sqbias[:])
                    if dsb_prev is not None:
                        for hi in range(2):
                            nc.tensor.matmul(
                                D[:], ident[:], dsb_prev[:, FD * hi:FD * (hi + 1)],
                                start=(ep == 1 and hi == 0), stop=False,
                            )
                    dsb_prev = dsb
                    for hi in range(2):
                        e = 2 * ep + hi
                        # broadcast edge e's crossing-row table across
                        # partitions via 0-partition-stride DMA from DRAM
                        qb = qbpool.tile([128, FD], bf16)
                        nc.sync.dma_start(qb[:], rhs_t_d[mi, e:e + 1, :].broadcast_to([128, FD]))
                        if e == 0:
                            nc.vector.tensor_scalar(cnt[:], qb[:], icol[:], None, ALU.is_gt)
                        else:
                            nc.vector.scalar_tensor_tensor(
                                cnt[:], qb[:], icol[:], cnt[:], ALU.is_gt, ALU.logical_xor
                            )
                for hi in range(2):
                    nc.tensor.matmul(
                        D[:], ident[:], dsb_prev[:, FD * hi:FD * (hi + 1)],
                        start=False, stop=(hi == 1),
                    )
                sqrt_insts.append(si)
                s = upool.tile([128, FD], f32, tag="s")
                nc.vector.tensor_scalar(s[:], cnt[:], 2.0, 1.0, ALU.mult, ALU.subtract)
                arg = argpool.tile([128, FD], f32)
                nc.vector.tensor_tensor(arg[:], s[:], D[:], ALU.mult)
                args.append(arg)
            for mi in range(IMGS):
                osb = opool.tile([128, FD], f32)
                sg = nc.scalar.activation(osb[:], args[mi][:], AFT.Sigmoid, scale=1.0 / (TAU * SIZE))
                if mi == 0:
                    # Keep ACT in the sqrt table-set until every sqrt is done:
                    # one Sqrt->Sigmoid table switch instead of thrashing.
                    for si in sqrt_insts:
                        bass._add_dep_helper(sg.ins, si.ins, False, "batch sigmoids after sqrts")
                nc.sync.dma_start(out_d[mi], osb[:])
    nc.compile()
    return nc


def _get_nc():
    global _NC_CACHE
    if _NC_CACHE is None:
        _NC_CACHE = _build_nc()
    return _NC_CACHE


def kernel(polygons: np.ndarray) -> np.ndarray:
    b, n, k, _ = polygons.shape
    in_maps = _host_tables(polygons)
    nc = _get_nc()
    res = run_bass_kernel_spmd(nc, in_maps, list(range(NCORES)))
    outs = []
    for c in range(NCORES):
        o = np.asarray(res.results[c]["out"])              # (IMGS, 128, 512)
        o = o.reshape(IMGS, 128, 2, 256).transpose(0, 2, 1, 3).reshape(IMGS, 256, 256)
        outs.append(o)
    full = np.concatenate(outs, axis=0)                     # (64, 256, 256)
    return full.reshape(b, n, SIZE, SIZE).astype(np.float32)


# revision 45
# speedup vs baseline: 1.2524x; 1.2524x over previous
"""Differentiable rasterization kernel for Trainium2 (8 NeuronCores, SPMD).

Problem: polygons (4,16,8,2) f32 in [0,1) -> out (4,16,256,256) f32
  out = sigmoid(C * D / (TAU*SIZE)) where
    D(i,j)  = sum_v ||(i,j) - vertex_v||           (sum of vertex distances)
    C(i,j)  = +1 inside polygon else -1            (even-odd rule)

Sharding: data-parallel over the 64 fused polygons, 8 per core.

Device-side strategy (per polygon image, [128 part = row%128, 512 free = (half,col)]):
  - PE (fp32r):  r2_v(i,j) = (i-vr)^2 + (j-vc)^2 as a K=3 matmul -> PSUM
  - ACT:         d_v = sqrt(r2_v)  PSUM->SBUF
  - PE (fp32r):  D += d_v  via identity-matmul accumulation into PSUM
  - PE (bf16):   term_e(i,j) = (n_e(j)+1) - i  (exact integers) -> PSUM
  - DVE:         parity ^= (term_e > 0)  fused scalar_tensor_tensor
  - GPSIMD:      arg = (2*parity-1) * D
  - ACT:         out = sigmoid(arg/512)   (single Sqrt->Sigmoid table switch)

Host-side prep is O(polygon-count): scanline crossing rows m_e(j) per column
(mirrors the reference f32 arithmetic), n = ceil(m)-1 so that the integer
comparison n+1 > i is exactly [m > i].
"""

import numpy as np
import ml_dtypes

import concourse.bass as bass
import concourse.bacc as bacc
import concourse.tile as tile
from concourse import mybir
from concourse.bass_utils import run_bass_kernel_spmd
from concourse._compat import get_trn_type

SIZE = 256
TAU = 2.0
NCORES = 8
IMGS = 8          # polygons per core
KV = 8            # vertices (= edges) per polygon
FD = 512          # free dim per image tile: (half h, col j)
ALU = mybir.AluOpType
AFT = mybir.ActivationFunctionType

_NC_CACHE = None


def _host_tables(polygons: np.ndarray):
    """Build per-core input tables. polygons: (4,16,8,2) f32."""
    b, n, k, _ = polygons.shape
    bn = b * n
    assert bn == NCORES * IMGS and k == KV
    # mirror reference: roll swaps (x,y)->(y,x), scale to pixels, f32 ops
    polys = (np.roll(polygons.astype(np.float32), 1, axis=-1)
             * np.float32(SIZE)).reshape(bn, k, 2)
    vr = polys[:, :, 0]            # row coords  (bn, k)
    vc = polys[:, :, 1]            # col coords  (bn, k)

    p = np.arange(128, dtype=np.float32)            # partition rows
    j = np.arange(SIZE, dtype=np.float32)           # columns
    h = np.array([0.0, 1.0], dtype=np.float32)      # row-half

    # ---- scanline crossings m_e(j): mirror reference _inside arithmetic ----
    x = vr                                           # (bn,k) edge start row
    y = vc                                           # (bn,k) edge start col
    xn = np.roll(vr, -1, axis=1)
    yn = np.roll(vc, -1, axis=1)
    pyj = j[None, None, :]                           # (1,1,256)
    cond = (y[:, :, None] > pyj) != (yn[:, :, None] > pyj)
    denom = np.where(cond, (yn - y)[:, :, None], np.float32(1.0)).astype(np.float32)
    # reference: xint = (xn - x) * (py - y) / denom + x  (f32 elementwise)
    num = ((xn - x)[:, :, None] * (pyj - y[:, :, None])).astype(np.float32)
    xint = (num / denom + x[:, :, None]).astype(np.float32)
    m = np.where(cond, xint, np.float32(-1e4))       # (bn, k, 256)
    # integer crossing row: [m > i]  <=>  [ceil(m)-1 >= i]  (exact, all m)
    nrow = np.clip(np.ceil(m) - 1.0, -1.0, 255.0).astype(np.float32)  # (bn,k,256)

    # constant rhs for the r2 matmul, centered to shrink fp32r rounding:
    # free layout f=(h,j), rows [(j-128)^2; (j-128); 1; w=128h-64]
    rhs_c = np.zeros((4, FD), dtype=np.float32)
    jc = j - np.float32(128.0)
    for hh in range(2):
        sl = slice(256 * hh, 256 * (hh + 1))
        rhs_c[0, sl] = jc * jc
        rhs_c[1, sl] = jc
        rhs_c[2, sl] = 1.0
        rhs_c[3, sl] = 128.0 * hh - 64.0
    icol = p.reshape(128, 1).copy()                  # per-partition row index

    in_maps = []
    for c in range(NCORES):
        g0 = c * IMGS
        # lhsT_r2[mi]: K=4 rows per vertex v, centered decomposition:
        #   r2 = (j-128)^2 - 2(vc-128)(j-128)
        #      + [(vc-128)^2 + (p+64-vr)^2 + 4096] + 2(p+64-vr)*(128h-64)
        #      = (j-vc)^2 + (p+128h-vr)^2
        lhsT_r2 = np.zeros((IMGS, 4, KV * 128), dtype=np.float32)
        # rhs_t[mi]: one q-row per edge: q(h,j) = n+1-128h (exact ints in bf16)
        rhs_t = np.zeros((IMGS, KV, FD), dtype=np.float32)
        for mi in range(IMGS):
            g = g0 + mi
            for v in range(KV):
                a = vc[g, v] - np.float32(128.0)
                u = (p + np.float32(64.0) - vr[g, v]).astype(np.float32)
                sl = slice(128 * v, 128 * (v + 1))
                lhsT_r2[mi, 0, sl] = 1.0
                lhsT_r2[mi, 1, sl] = np.float32(-2.0) * a
                lhsT_r2[mi, 2, sl] = a * a + u * u + np.float32(4096.0)
                lhsT_r2[mi, 3, sl] = np.float32(2.0) * u
                q = nrow[g, v] + 1.0
                for hh in range(2):
                    rhs_t[mi, v, 256 * hh:256 * (hh + 1)] = q - 128.0 * hh
        in_maps.append({
            "lhsT_r2": lhsT_r2,
            "rhs_t": rhs_t.astype(ml_dtypes.bfloat16),
            "rhs_c": rhs_c,
            "icol": icol,
            "ident": np.eye(128, dtype=np.float32).astype(ml_dtypes.bfloat16),
        })
    return in_maps


def _build_nc():
    f32 = mybir.dt.float32
    f32r = mybir.dt.float32r
    bf16 = mybir.dt.bfloat16

    nc = bacc.Bacc(
        get_trn_type(),
        target_bir_lowering=False,
        debug=False,
        num_devices=NCORES,
    )
    lhsT_r2_d = nc.declare_dram_parameter("lhsT_r2", [IMGS, 4, KV * 128], f32r, isOutput=False)
    rhs_t_d = nc.declare_dram_parameter("rhs_t", [IMGS, KV, FD], bf16, isOutput=False)
    rhs_c_d = nc.declare_dram_parameter("rhs_c", [4, FD], f32r, isOutput=False)
    icol_d = nc.declare_dram_parameter("icol", [128, 1], f32, isOutput=False)
    ident_d = nc.declare_dram_parameter("ident", [128, 128], bf16, isOutput=False)
    out_d = nc.declare_dram_parameter("out", [IMGS, 128, FD], f32, isOutput=True)

    with tile.TileContext(nc) as tc:
        with (
            tc.tile_pool(name="const", bufs=1) as cpool,
            tc.tile_pool(name="tabs", bufs=2) as tabpool,
            tc.tile_pool(name="r2", bufs=3, space="PSUM") as r2pool,      # 3x2 banks (pairs)
            tc.tile_pool(name="Dacc", bufs=2, space="PSUM") as dpool,     # 2 banks
            tc.tile_pool(name="qb", bufs=4) as qbpool,                    # SBUF bf16
            tc.tile_pool(name="dsb", bufs=3) as dsbpool,
            tc.tile_pool(name="cnt", bufs=2) as cntpool,
            tc.tile_pool(name="Dsb", bufs=2) as Dsbpool,
            tc.tile_pool(name="u", bufs=2) as upool,
            tc.tile_pool(name="arg", bufs=IMGS) as argpool,
            tc.tile_pool(name="osb", bufs=3) as opool,
        ):
            rhs_c = cpool.tile([4, FD], f32r)
            icol = cpool.tile([128, 1], f32)
            ident = cpool.tile([128, 128], bf16)
            sqbias = cpool.tile([128, 1], f32)
            nc.sync.dma_start(rhs_c[:], rhs_c_d[:])
            nc.sync.dma_start(icol[:], icol_d[:])
            nc.sync.dma_start(ident[:], ident_d[:])
            nc.gpsimd.memset(sqbias[:], 8.0)

            args = []
            sqrt_insts = []
            for mi in range(IMGS):
                lhsT_r2 = tabpool.tile([4, KV * 128], f32r, tag="lhsT_r2")
                nc.sync.dma_start(lhsT_r2[:], lhsT_r2_d[mi])
                D = dpool.tile([128, FD], f32)
                cnt = cntpool.tile([128, FD], f32, tag="cntA")
                # pair vertices: two r2 matmuls fill one [128,1024] PSUM tile,
                # one wide sqrt per pair; D-accumulation matmuls are emitted
                # one pair late so PE never head-of-line blocks on ACT.
                dsb_prev = None
                for ep in range(KV // 2):
                    r2 = r2pool.tile([128, 2 * FD], f32)
                    dsb = dsbpool.tile([128, 2 * FD], bf16)
                    for hi in range(2):
                        e = 2 * ep + hi
                        nc.tensor.matmul(
                            r2[:, FD * hi:FD * (hi + 1)],
                            lhsT_r2[:, 128 * e:128 * (e + 1)],
                            rhs_c[:],
                            start=True, stop=True,
                        )
                    # bias guards fp32r matmul rounding driving r2 slightly
                    # negative near a vertex (sqrt(neg)=NaN)
                    si = nc.scalar.activation(dsb[:], r2[:], AFT.Sqrt, bias=sqbias[:])
                    if dsb_prev is not None:
                        for hi in range(2):
                            nc.tensor.matmul(
                                D[:], ident[:], dsb_prev[:, FD * hi:FD * (hi + 1)],
                                start=(ep == 1 and hi == 0), stop=(ep == KV // 2 - 1 and hi == 1),
                            )
                    dsb_prev = dsb
                    # broadcast the pair's two crossing-row tables across
                    # partitions in one 0-partition-stride DMA from DRAM
                    qb = qbpool.tile([128, 2 * FD], bf16)
                    nc.sync.dma_start(
                        qb[:], rhs_t_d[mi, 2 * ep:2 * ep + 2, :].rearrange("e f -> (e f)")[None, :].broadcast_to([128, 2 * FD]))
                    for hi in range(2):
                        e = 2 * ep + hi
                        qbe = qb[:, FD * hi:FD * (hi + 1)]
                        if e == 0:
                            nc.vector.tensor_scalar(cnt[:], qbe, icol[:], None, ALU.is_gt)
                        else:
                            nc.vector.scalar_tensor_tensor(
                                cnt[:], qbe, icol[:], cnt[:], ALU.is_gt, ALU.logical_xor
                            )
                # last pair's D contribution on DVE (bf16 2x add) instead of PE
                dd = upool.tile([128, FD], bf16, tag="dd")
                nc.vector.tensor_tensor(dd[:], dsb_prev[:, :FD], dsb_prev[:, FD:], ALU.add)
                sqrt_insts.append(si)
                s = upool.tile([128, FD], f32, tag="s")
                nc.vector.tensor_scalar(s[:], cnt[:], 2.0, 1.0, ALU.mult, ALU.subtract)
                t = upool.tile([128, FD], f32, tag="t")
                nc.vector.scalar_tensor_tensor(t[:], dd[:], 0.0, D[:], ALU.bypass, ALU.add)
                arg = argpool.tile([128, FD], f32)
                nc.vector.tensor_tensor(arg[:], s[:], t[:], ALU.mult)
                args.append(arg)
            for mi in range(IMGS):
                osb = opool.tile([128, FD], f32)
                sg = nc.scalar.activation(osb[:], args[mi][:], AFT.Sigmoid, scale=1.0 / (TAU * SIZE))
                if mi == 0:
                    # Keep ACT in the sqrt table-set until every sqrt is done:
                    # one Sqrt->Sigmoid table switch instead of thrashing.
                    for si in sqrt_insts:
                        bass._add_dep_helper(sg.ins, si.ins, False, "batch sigmoids after sqrts")
                nc.sync.dma_start(out_d[mi], osb[:])
    nc.compile()
    return nc


def _get_nc():
    global _NC_CACHE
    if _NC_CACHE is None:
        _NC_CACHE = _build_nc()
    return _NC_CACHE


def kernel(polygons: np.ndarray) -> np.ndarray:
    b, n, k, _ = polygons.shape
    in_maps = _host_tables(polygons)
    nc = _get_nc()
    res = run_bass_kernel_spmd(nc, in_maps, list(range(NCORES)))
    outs = []
    for c in range(NCORES):
        o = np.asarray(res.results[c]["out"])              # (IMGS, 128, 512)
        o = o.reshape(IMGS, 128, 2, 256).transpose(0, 2, 1, 3).reshape(IMGS, 256, 256)
        outs.append(o)
    full = np.concatenate(outs, axis=0)                     # (64, 256, 256)
    return full.reshape(b, n, SIZE, SIZE).astype(np.float32)
